# revision 25
# baseline (speedup 1.0000x reference)
"""Mutual channel attention (sparse_attention) TRN2 Bass kernel.

Problem: x1, x2 of shape (16, 512, 64, 64) fp32.
  q = x1.reshape(B, C, D), k = x2.reshape(B, C, D), D = 4096, scale = 1/64
  S    = q @ k^T * scale                      [B, 512, 512]
  outA = softmax_rows(S) @ k                  -> (16, 512, 64, 64)
  outB = softmax_rows(S^T) @ q                -> (16, 512, 64, 64)

Key algebra: without max-subtraction (scores ~ N(0,1), safe here),
P = exp(S*scale) serves BOTH directions; only the normalization sums
differ (row sums of P for A, column sums of P for B).

Sharding: pure data parallel, 2 batches per core across 8 cores.

Everything runs in fp16 (inputs quantize at ~2^-11 RMS, far inside the
2e-2 gate).  The host ships each batch's q and k in BOTH layouts --
original [C, D] and transposed [D, C] -- as fp16.  That costs the same
HBM bytes as fp32 single-layout but removes ALL 256 per-batch q/k PE
transposes of the fp32r version:

  scores  S_ce[cc] = sum_dc  qT[dc][:,cc*128:+128]^T @ kT[dc]
                     (both operands d-on-partitions, 4x32 matmuls N=512)
  outA[cc,g]       = sum_ec  P_ec[ec][:,cc*128:+128]^T @ k[ec][:,g*512:+512]
  outB[ec,g]       = sum_cc  P_ce[cc][:,ec*128:+128]^T @ q[cc][:,g*512:+512]

P_ce = exp(S*1/64) with fused row sums (direction A); P_ec from 16
128x128 PE transposes with column sums fused on the PSUM->SBUF copy
(direction B).  Outputs are written fp16 (host upconverts).

DMA trigger instructions (DMA_DIRECT2D) cost ~0.6us of issuing-engine
time each, so DMAs are batched 512KB per trigger: transposed loads come
as quad-d-chunk tiles [128, 4x512] (host pre-tiles the DRAM layout so
each is one contiguous 2D transfer) and stores go out as [128, 4x512]
supertiles into a tiled DRAM layout (host untiles).  Loads issue on the
Sync HWDGE queue in consumption order; stores ride the Scalar HWDGE
queue so not-yet-ready stores never head-of-line-block prefetch loads
(queues share the DMA engines without priority, so loads must keep a
single queue to preserve their ordering).

The out phase runs B0 B1 A0 B2 A1 ... B7 A6 A7: outB groups depend only
on exp output, so the first outA group starts ~7us after scores end,
past the ~5.4us of serial ACT work producing the P_ec copies.

PSUM: 4 score banks (reused as transpose staging, then as half of the
8-deep out-accumulation ring) + 4 dedicated out banks.

Measured: 191.3us HW exec (slowest core) vs 285.6us fp32r baseline;
PE ~90% busy at the 216ns/matmul streaming rate; rel err 4.2e-4.
"""

import numpy as np

B, C, D = 16, 512, 4096
N_CORES = 8
B_PER_CORE = B // N_CORES  # 2
CC = C // 128  # 4 c-chunks
DC = D // 128  # 32 d-chunks
NQ = DC // 4  # 8 quad-chunk load tiles per tensor per batch
NG = D // 512  # 8 d-groups of 512 in the out phase

_COMPILED = {}


def _build():
    import concourse.mybir as mybir
    from concourse import bacc, tile

    f32 = mybir.dt.float32
    f16 = mybir.dt.float16
    AF = mybir.ActivationFunctionType
    ROWS = B_PER_CORE * C  # 1024
    QROWS = B_PER_CORE * NQ * 128  # 2048 rows of quad-tiled qT/kT
    OROWS = B_PER_CORE * NG * 128  # 2048 rows of tiled outputs

    nc = bacc.Bacc(None, target_bir_lowering=False)
    # qT/kT tiled: row (b*NQ + j)*128 + p, col s*512 + c  <->  q^T[b, (j*4+s)*128+p, c]
    qT = nc.declare_dram_parameter("qT", [QROWS, 2048], f16, isOutput=False)
    kT = nc.declare_dram_parameter("kT", [QROWS, 2048], f16, isOutput=False)
    # batch 0's first d-quad as four contiguous singles (row s*128+p, col c):
    # the cold DMA path delivers the first 256KB ~4x sooner than a 1MB quad
    qTs = nc.declare_dram_parameter("qTs", [512, 512], f16, isOutput=False)
    kTs = nc.declare_dram_parameter("kTs", [512, 512], f16, isOutput=False)
    qO = nc.declare_dram_parameter("qO", [ROWS, D], f16, isOutput=False)
    kO = nc.declare_dram_parameter("kO", [ROWS, D], f16, isOutput=False)
    ident = nc.declare_dram_parameter("ident", [128, 128], f16, isOutput=False)
    # outputs tiled: row (b*NG + g)*128 + p, col cc*512 + c  <->  out[b, cc*128+p, g*512+c]
    outA = nc.declare_dram_parameter("outA", [OROWS, 2048], f16, isOutput=True)
    outB = nc.declare_dram_parameter("outB", [OROWS, 2048], f16, isOutput=True)

    with tile.TileContext(nc) as tc:
        with (
            tc.tile_pool(name="const", bufs=1) as constp,
            tc.tile_pool(name="qkT", bufs=1) as qkt,
            tc.tile_pool(name="qkO", bufs=1) as qko,
            tc.tile_pool(name="pp", bufs=1) as pp,
            tc.tile_pool(name="rp", bufs=2) as rp,
            tc.tile_pool(name="osb", bufs=3) as osb,
            tc.tile_pool(name="sps", bufs=1, space="PSUM") as sps,
            tc.tile_pool(name="ops", bufs=4, space="PSUM") as ops,
        ):
            # deferred per-batch qT/kT quad loads: batch 0's run up front;
            # batch b+1's are interleaved into batch b's out phase.
            qTt = [[None] * NQ for _ in range(B_PER_CORE)]
            kTt = [[None] * NQ for _ in range(B_PER_CORE)]

            def t_load(b, j):
                rows = slice((b * NQ + j) * 128, (b * NQ + j + 1) * 128)
                qt = qkt.tile([128, 2048], f16, tag=f"qT{j}", name=f"qT{j}")
                kt = qkt.tile([128, 2048], f16, tag=f"kT{j}", name=f"kT{j}")
                nc.sync.dma_start(qt[:], qT[rows, :])
                nc.sync.dma_start(kt[:], kT[rows, :])
                qTt[b][j] = qt
                kTt[b][j] = kt

            # batch 0, quad 0: four 128KB singles so the scores phase can
            # start as soon as the first 256KB lands.  Their triggers are
            # interleaved with the first quad triggers -- 8 singles up
            # front would delay the quad stream by ~5us of trigger time.
            qSt, kSt = [], []

            def s_load(s):
                qs = qkt.tile([128, 512], f16, tag=f"qS{s}", name=f"qS{s}")
                ks = qkt.tile([128, 512], f16, tag=f"kS{s}", name=f"kS{s}")
                nc.sync.dma_start(qs[:], qTs[s * 128 : (s + 1) * 128, :])
                nc.sync.dma_start(ks[:], kTs[s * 128 : (s + 1) * 128, :])
                qSt.append(qs)
                kSt.append(ks)

            s_load(0)
            s_load(1)
            idt = constp.tile([128, 128], f16, name="idt")
            nc.sync.dma_start(idt[:], ident[:])
            t_load(0, 1)
            s_load(2)
            t_load(0, 2)
            s_load(3)
            for j in range(3, NQ):
                t_load(0, j)

            for b in range(B_PER_CORE):
                r0 = b * C

                # ---- q/k original-layout loads (needed by out phase) ----
                qot, kot = [], []
                for cc in range(CC):
                    rows = slice(r0 + cc * 128, r0 + (cc + 1) * 128)
                    qo = qko.tile([128, D], f16, tag=f"qo{cc}", name=f"qo{cc}")
                    ko = qko.tile([128, D], f16, tag=f"ko{cc}", name=f"ko{cc}")
                    nc.sync.dma_start(qo[:], qO[rows, :])
                    nc.sync.dma_start(ko[:], kO[rows, :])
                    qot.append(qo)
                    kot.append(ko)

                # ---- scores: S_ce[cc] accumulates over 32 d-chunks ----
                s_ps = [
                    sps.tile([128, C], f32, tag=f"s{cc}", name=f"s{cc}")
                    for cc in range(CC)
                ]
                for dc in range(DC):
                    j, s = divmod(dc, 4)
                    if b == 0 and j == 0:
                        mv = kSt[s][:]
                        st = lambda cc: qSt[s][:, cc * 128 : (cc + 1) * 128]
                    else:
                        mv = kTt[b][j][:, s * 512 : (s + 1) * 512]
                        st = lambda cc: qTt[b][j][
                            :, s * 512 + cc * 128 : s * 512 + (cc + 1) * 128
                        ]
                    for cc in range(CC):
                        nc.tensor.matmul(
                            s_ps[cc][:],
                            st(cc),
                            mv,
                            start=(dc == 0),
                            stop=(dc == DC - 1),
                        )

                # ---- exp + row sums (direction A) ----
                p_ce = []
                rinv_a = []
                for cc in range(CC):
                    p = pp.tile([128, C], f16, tag=f"pce{cc}", name=f"pce{cc}")
                    rs = rp.tile([128, 1], f32, tag=f"rsa{cc}", name=f"rsa{cc}")
                    nc.scalar.activation(
                        p[:], s_ps[cc][:], AF.Exp, scale=1.0 / 64.0, accum_out=rs[:]
                    )
                    ri = rp.tile([128, 1], f32, tag=f"ria{cc}", name=f"ria{cc}")
                    nc.vector.reciprocal(ri[:], rs[:])
                    p_ce.append(p)
                    rinv_a.append(ri)

                # ---- transpose P -> P_ec + column sums (direction B) ----
                # staging reuses the score banks (freed by exp); cc-outer so
                # transposes of P_ce[cc] start as soon as exp[cc] lands.
                stg = [
                    sps.tile([128, C], f16, tag=f"s{ec}", name=f"stg{ec}")
                    for ec in range(CC)
                ]
                for cc in range(CC):
                    for ec in range(CC):
                        nc.tensor.transpose(
                            stg[ec][:, cc * 128 : (cc + 1) * 128],
                            p_ce[cc][:, ec * 128 : (ec + 1) * 128],
                            idt[:],
                        )
                p_ec = []
                rinv_b = []
                for ec in range(CC):
                    p = pp.tile([128, C], f16, tag=f"pec{ec}", name=f"pec{ec}")
                    rs = rp.tile([128, 1], f32, tag=f"rsb{ec}", name=f"rsb{ec}")
                    nc.scalar.activation(p[:], stg[ec][:], AF.Copy, accum_out=rs[:])
                    ri = rp.tile([128, 1], f32, tag=f"rib{ec}", name=f"rib{ec}")
                    nc.vector.reciprocal(ri[:], rs[:])
                    p_ec.append(p)
                    rinv_b.append(ri)

                # ---- out phase: 64 groups of 4 accumulating matmuls ----
                # PSUM ring 8 deep: 4 "o" slots + the 4 score banks.
                gi = 0

                def out_psum(name):
                    nonlocal gi
                    if gi % 8 < 4:
                        t = ops.tile([128, 512], f32, tag="o", name=name)
                    else:
                        t = sps.tile([128, 512], f32, tag=f"s{gi % 4}", name=name)
                    gi += 1
                    return t

                def do_b_group(g, store_eng=None):
                    gsl = slice(g * 512, (g + 1) * 512)
                    orow = slice((b * NG + g) * 128, (b * NG + g + 1) * 128)
                    ob4 = osb.tile([128, 2048], f16, tag="ob", name="ob_sb")
                    for ec in range(CC):  # outB rows ec*128..+128
                        o_ps = out_psum("ob_ps")
                        for cc in range(CC):
                            nc.tensor.matmul(
                                o_ps[:],
                                p_ce[cc][:, ec * 128 : (ec + 1) * 128],
                                qot[cc][:, gsl],
                                start=(cc == 0),
                                stop=(cc == CC - 1),
                            )
                        osl = ob4[:, ec * 512 : (ec + 1) * 512]
                        if ec % 2 == 0:
                            nc.vector.tensor_scalar_mul(osl, o_ps[:], rinv_b[ec][:])
                        else:
                            nc.scalar.activation(
                                osl, o_ps[:], AF.Copy, scale=rinv_b[ec][:]
                            )
                    (store_eng or nc.scalar).dma_start(outB[orow, :], ob4[:])

                def do_a_group(g, store_eng=None):
                    gsl = slice(g * 512, (g + 1) * 512)
                    orow = slice((b * NG + g) * 128, (b * NG + g + 1) * 128)
                    oa4 = osb.tile([128, 2048], f16, tag="oa", name="oa_sb")
                    for cc in range(CC):  # outA rows cc*128..+128
                        o_ps = out_psum("oa_ps")
                        for ec in range(CC):
                            nc.tensor.matmul(
                                o_ps[:],
                                p_ec[ec][:, cc * 128 : (cc + 1) * 128],
                                kot[ec][:, gsl],
                                start=(ec == 0),
                                stop=(ec == CC - 1),
                            )
                        osl = oa4[:, cc * 512 : (cc + 1) * 512]
                        if cc % 2 == 0:
                            nc.vector.tensor_scalar_mul(osl, o_ps[:], rinv_a[cc][:])
                        else:
                            nc.scalar.activation(
                                osl, o_ps[:], AF.Copy, scale=rinv_a[cc][:]
                            )
                    (store_eng or nc.scalar).dma_start(outA[orow, :], oa4[:])

                # outB groups only need P_ce (exp output); outA needs the
                # P_ec copies, ~5.4us of serial ACT work after scores end.
                # Run TWO B-groups before the first A-group so the out
                # phase never waits on ACT: B0 B1 A0 B2 A1 ... B7 A6 A7.
                tail = b == B_PER_CORE - 1
                do_b_group(0)
                for g in range(1, NG):
                    do_b_group(g, nc.sync if (tail and g == NG - 1) else None)
                    do_a_group(g - 1)
                    # spread next batch's transposed-layout loads across
                    # this batch's out phase (one quad pair per BA pair)
                    if b + 1 < B_PER_CORE:
                        t_load(b + 1, g - 1)
                do_a_group(NG - 1, nc.sync if tail else None)
                if b + 1 < B_PER_CORE:
                    t_load(b + 1, NG - 1)

    nc.finalize()
    return nc


def _get_nc():
    if "nc" not in _COMPILED:
        _COMPILED["nc"] = _build()
    return _COMPILED["nc"]


def build_in_maps(x1: np.ndarray, x2: np.ndarray):
    """Host-side shard + layout prep: fp16, tiled transposed + original."""
    Xq = np.asarray(x1, dtype=np.float32).reshape(B, C, D).astype(np.float16)
    Xk = np.asarray(x2, dtype=np.float32).reshape(B, C, D).astype(np.float16)
    ident = np.eye(128, dtype=np.float16)

    def tiled_T(Xb):
        # [bpc, C, D] -> transposed [bpc, D, C] -> quad-tiled [QROWS, 2048]
        # row (b*NQ + j)*128 + p, col s*512 + c  <->  T[b, (j*4+s)*128 + p, c]
        T = Xb.transpose(0, 2, 1).reshape(B_PER_CORE, NQ, 4, 128, C)
        return np.ascontiguousarray(T.transpose(0, 1, 3, 2, 4)).reshape(
            B_PER_CORE * NQ * 128, 4 * C
        )

    def singles0(Xb):
        # batch 0's first d-quad as 4 stacked [128, C] singles:
        # row s*128 + p, col c  <->  T[0, s*128 + p, c]  (d = s*128+p < 512)
        return np.ascontiguousarray(Xb[0, :, 0:512].T)

    in_maps = []
    for i in range(N_CORES):
        sl = slice(i * B_PER_CORE, (i + 1) * B_PER_CORE)
        in_maps.append(
            {
                "qT": tiled_T(Xq[sl]),
                "kT": tiled_T(Xk[sl]),
                "qTs": singles0(Xq[sl]),
                "kTs": singles0(Xk[sl]),
                "qO": Xq[sl].reshape(B_PER_CORE * C, D),
                "kO": Xk[sl].reshape(B_PER_CORE * C, D),
                "ident": ident,
            }
        )
    return in_maps


def _untile_out(arr):
    # [OROWS, 2048] -> [bpc, C, D]: arr[(b*NG+g)*128+p, cc*512+c] = out[b, cc*128+p, g*512+c]
    t = arr.reshape(B_PER_CORE, NG, 128, CC, 512).transpose(0, 3, 2, 1, 4)
    return t.reshape(B_PER_CORE, C, D)


def kernel(x1: np.ndarray, x2: np.ndarray):
    from concourse.bass_utils import run_bass_kernel_spmd

    nc = _get_nc()
    in_maps = build_in_maps(x1, x2)

    res = None
    for attempt in range(3):
        try:
            res = run_bass_kernel_spmd(nc, in_maps, list(range(N_CORES))).results
            break
        except Exception:
            if attempt == 2:
                raise
    assert res is not None

    outA = np.empty((B, C, 64, 64), dtype=np.float32)
    outB = np.empty((B, C, 64, 64), dtype=np.float32)
    for i in range(N_CORES):
        sl = slice(i * B_PER_CORE, (i + 1) * B_PER_CORE)
        outA[sl] = _untile_out(res[i]["outA"]).astype(np.float32).reshape(
            B_PER_CORE, C, 64, 64
        )
        outB[sl] = _untile_out(res[i]["outB"]).astype(np.float32).reshape(
            B_PER_CORE, C, 64, 64
        )
    return outA, outB


# revision 27
# speedup vs baseline: 1.0074x; 1.0074x over previous
"""Mutual channel attention (sparse_attention) TRN2 Bass kernel.

Problem: x1, x2 of shape (16, 512, 64, 64) fp32.
  q = x1.reshape(B, C, D), k = x2.reshape(B, C, D), D = 4096, scale = 1/64
  S    = q @ k^T * scale                      [B, 512, 512]
  outA = softmax_rows(S) @ k                  -> (16, 512, 64, 64)
  outB = softmax_rows(S^T) @ q                -> (16, 512, 64, 64)

Key algebra: without max-subtraction (scores ~ N(0,1), safe here),
P = exp(S*scale) serves BOTH directions; only the normalization sums
differ (row sums of P for A, column sums of P for B).

Sharding: pure data parallel, 2 batches per core across 8 cores.

Everything runs in fp16 (inputs quantize at ~2^-11 RMS, far inside the
2e-2 gate).  The host ships each batch's q and k in BOTH layouts --
original [C, D] and transposed [D, C] -- as fp16.  That costs the same
HBM bytes as fp32 single-layout but removes ALL 256 per-batch q/k PE
transposes of the fp32r version:

  scores  S_ce[cc] = sum_dc  qT[dc][:,cc*128:+128]^T @ kT[dc]
                     (both operands d-on-partitions, 4x32 matmuls N=512)
  outA[cc,g]       = sum_ec  P_ec[ec][:,cc*128:+128]^T @ k[ec][:,g*512:+512]
  outB[ec,g]       = sum_cc  P_ce[cc][:,ec*128:+128]^T @ q[cc][:,g*512:+512]

P_ce = exp(S*1/64) with fused row sums (direction A); P_ec from 16
128x128 PE transposes with column sums fused on the PSUM->SBUF copy
(direction B).  Outputs are written fp16 (host upconverts).

DMA trigger instructions (DMA_DIRECT2D) cost ~0.6us of issuing-engine
time each, so DMAs are batched 512KB per trigger: transposed loads come
as quad-d-chunk tiles [128, 4x512] (host pre-tiles the DRAM layout so
each is one contiguous 2D transfer) and stores go out as [128, 4x512]
supertiles into a tiled DRAM layout (host untiles).  Loads issue on the
Sync HWDGE queue, stores on the Scalar HWDGE queue so not-yet-ready
stores never head-of-line-block prefetch loads.

PSUM: 4 score banks (reused as transpose staging, then as half of the
8-deep out-accumulation ring) + 4 dedicated out banks.
"""

import numpy as np

B, C, D = 16, 512, 4096
N_CORES = 8
B_PER_CORE = B // N_CORES  # 2
CC = C // 128  # 4 c-chunks
DC = D // 128  # 32 d-chunks
NQ = DC // 4  # 8 quad-chunk load tiles per tensor per batch
NG = D // 512  # 8 d-groups of 512 in the out phase

_COMPILED = {}


def _build():
    import concourse.mybir as mybir
    from concourse import bacc, tile

    f32 = mybir.dt.float32
    f16 = mybir.dt.float16
    AF = mybir.ActivationFunctionType
    ROWS = B_PER_CORE * C  # 1024
    QROWS = B_PER_CORE * NQ * 128  # 2048 rows of quad-tiled qT/kT
    OROWS = B_PER_CORE * NG * 128  # 2048 rows of tiled outputs

    nc = bacc.Bacc(None, target_bir_lowering=False)
    # qT/kT tiled: row (b*NQ + j)*128 + p, col s*512 + c  <->  q^T[b, (j*4+s)*128+p, c]
    qT = nc.declare_dram_parameter("qT", [QROWS, 2048], f16, isOutput=False)
    kT = nc.declare_dram_parameter("kT", [QROWS, 2048], f16, isOutput=False)
    qO = nc.declare_dram_parameter("qO", [ROWS, D], f16, isOutput=False)
    kO = nc.declare_dram_parameter("kO", [ROWS, D], f16, isOutput=False)
    ident = nc.declare_dram_parameter("ident", [128, 128], f16, isOutput=False)
    # outputs tiled: row (b*NG + g)*128 + p, col cc*512 + c  <->  out[b, cc*128+p, g*512+c]
    outA = nc.declare_dram_parameter("outA", [OROWS, 2048], f16, isOutput=True)
    outB = nc.declare_dram_parameter("outB", [OROWS, 2048], f16, isOutput=True)

    with tile.TileContext(nc) as tc:
        with (
            tc.tile_pool(name="const", bufs=1) as constp,
            tc.tile_pool(name="qkT", bufs=1) as qkt,
            tc.tile_pool(name="qkO", bufs=1) as qko,
            tc.tile_pool(name="pp", bufs=1) as pp,
            tc.tile_pool(name="rp", bufs=2) as rp,
            tc.tile_pool(name="osb", bufs=3) as osb,
            tc.tile_pool(name="sps", bufs=1, space="PSUM") as sps,
            tc.tile_pool(name="ops", bufs=4, space="PSUM") as ops,
        ):
            # deferred per-batch qT/kT quad loads: batch 0's run up front;
            # batch b+1's are interleaved into batch b's out phase.
            qTt = [[None] * NQ for _ in range(B_PER_CORE)]
            kTt = [[None] * NQ for _ in range(B_PER_CORE)]

            def t_load(b, j, k_eng=None):
                rows = slice((b * NQ + j) * 128, (b * NQ + j + 1) * 128)
                qt = qkt.tile([128, 2048], f16, tag=f"qT{j}", name=f"qT{j}")
                kt = qkt.tile([128, 2048], f16, tag=f"kT{j}", name=f"kT{j}")
                nc.sync.dma_start(qt[:], qT[rows, :])
                (k_eng or nc.sync).dma_start(kt[:], kT[rows, :])
                qTt[b][j] = qt
                kTt[b][j] = kt

            # batch 0 cold start: kT quads ride the (idle until ~45us)
            # Scalar queue so the q and k streams trigger in parallel --
            # identical deadlines, so fair engine-sharing matches the
            # scores phase's 1:1 consumption ratio.  Batch 1's loads keep
            # the single sync queue (scalar is pacing stores by then).
            idt = None
            for j in range(NQ):
                t_load(0, j, k_eng=nc.scalar)
                if j == 0:
                    idt = constp.tile([128, 128], f16, name="idt")
                    nc.sync.dma_start(idt[:], ident[:])

            for b in range(B_PER_CORE):
                r0 = b * C

                # ---- q/k original-layout loads (needed by out phase) ----
                qot, kot = [], []
                for cc in range(CC):
                    rows = slice(r0 + cc * 128, r0 + (cc + 1) * 128)
                    qo = qko.tile([128, D], f16, tag=f"qo{cc}", name=f"qo{cc}")
                    ko = qko.tile([128, D], f16, tag=f"ko{cc}", name=f"ko{cc}")
                    nc.sync.dma_start(qo[:], qO[rows, :])
                    nc.sync.dma_start(ko[:], kO[rows, :])
                    qot.append(qo)
                    kot.append(ko)

                # ---- scores: S_ce[cc] accumulates over 32 d-chunks ----
                s_ps = [
                    sps.tile([128, C], f32, tag=f"s{cc}", name=f"s{cc}")
                    for cc in range(CC)
                ]
                for dc in range(DC):
                    j, s = divmod(dc, 4)
                    mv = kTt[b][j][:, s * 512 : (s + 1) * 512]
                    for cc in range(CC):
                        nc.tensor.matmul(
                            s_ps[cc][:],
                            qTt[b][j][:, s * 512 + cc * 128 : s * 512 + (cc + 1) * 128],
                            mv,
                            start=(dc == 0),
                            stop=(dc == DC - 1),
                        )

                # ---- exp + row sums (direction A) ----
                p_ce = []
                rinv_a = []
                for cc in range(CC):
                    p = pp.tile([128, C], f16, tag=f"pce{cc}", name=f"pce{cc}")
                    rs = rp.tile([128, 1], f32, tag=f"rsa{cc}", name=f"rsa{cc}")
                    nc.scalar.activation(
                        p[:], s_ps[cc][:], AF.Exp, scale=1.0 / 64.0, accum_out=rs[:]
                    )
                    ri = rp.tile([128, 1], f32, tag=f"ria{cc}", name=f"ria{cc}")
                    nc.vector.reciprocal(ri[:], rs[:])
                    p_ce.append(p)
                    rinv_a.append(ri)

                # ---- transpose P -> P_ec + column sums (direction B) ----
                # staging reuses the score banks (freed by exp); cc-outer so
                # transposes of P_ce[cc] start as soon as exp[cc] lands.
                stg = [
                    sps.tile([128, C], f16, tag=f"s{ec}", name=f"stg{ec}")
                    for ec in range(CC)
                ]
                for cc in range(CC):
                    for ec in range(CC):
                        nc.tensor.transpose(
                            stg[ec][:, cc * 128 : (cc + 1) * 128],
                            p_ce[cc][:, ec * 128 : (ec + 1) * 128],
                            idt[:],
                        )
                p_ec = []
                rinv_b = []
                for ec in range(CC):
                    p = pp.tile([128, C], f16, tag=f"pec{ec}", name=f"pec{ec}")
                    rs = rp.tile([128, 1], f32, tag=f"rsb{ec}", name=f"rsb{ec}")
                    nc.scalar.activation(p[:], stg[ec][:], AF.Copy, accum_out=rs[:])
                    ri = rp.tile([128, 1], f32, tag=f"rib{ec}", name=f"rib{ec}")
                    nc.vector.reciprocal(ri[:], rs[:])
                    p_ec.append(p)
                    rinv_b.append(ri)

                # ---- out phase: 64 groups of 4 accumulating matmuls ----
                # PSUM ring 8 deep: 4 "o" slots + the 4 score banks.
                gi = 0

                def out_psum(name):
                    nonlocal gi
                    if gi % 8 < 4:
                        t = ops.tile([128, 512], f32, tag="o", name=name)
                    else:
                        t = sps.tile([128, 512], f32, tag=f"s{gi % 4}", name=name)
                    gi += 1
                    return t

                def do_b_group(g):
                    gsl = slice(g * 512, (g + 1) * 512)
                    orow = slice((b * NG + g) * 128, (b * NG + g + 1) * 128)
                    ob4 = osb.tile([128, 2048], f16, tag="ob", name="ob_sb")
                    for ec in range(CC):  # outB rows ec*128..+128
                        o_ps = out_psum("ob_ps")
                        for cc in range(CC):
                            nc.tensor.matmul(
                                o_ps[:],
                                p_ce[cc][:, ec * 128 : (ec + 1) * 128],
                                qot[cc][:, gsl],
                                start=(cc == 0),
                                stop=(cc == CC - 1),
                            )
                        osl = ob4[:, ec * 512 : (ec + 1) * 512]
                        if ec % 2 == 0:
                            nc.vector.tensor_scalar_mul(osl, o_ps[:], rinv_b[ec][:])
                        else:
                            nc.scalar.activation(
                                osl, o_ps[:], AF.Copy, scale=rinv_b[ec][:]
                            )
                    nc.scalar.dma_start(outB[orow, :], ob4[:])

                def do_a_group(g):
                    gsl = slice(g * 512, (g + 1) * 512)
                    orow = slice((b * NG + g) * 128, (b * NG + g + 1) * 128)
                    oa4 = osb.tile([128, 2048], f16, tag="oa", name="oa_sb")
                    for cc in range(CC):  # outA rows cc*128..+128
                        o_ps = out_psum("oa_ps")
                        for ec in range(CC):
                            nc.tensor.matmul(
                                o_ps[:],
                                p_ec[ec][:, cc * 128 : (cc + 1) * 128],
                                kot[ec][:, gsl],
                                start=(ec == 0),
                                stop=(ec == CC - 1),
                            )
                        osl = oa4[:, cc * 512 : (cc + 1) * 512]
                        if cc % 2 == 0:
                            nc.vector.tensor_scalar_mul(osl, o_ps[:], rinv_a[cc][:])
                        else:
                            nc.scalar.activation(
                                osl, o_ps[:], AF.Copy, scale=rinv_a[cc][:]
                            )
                    nc.scalar.dma_start(outA[orow, :], oa4[:])

                # outB groups only need P_ce (exp output); outA needs the
                # P_ec copies, ~5.4us of serial ACT work after scores end.
                # Run TWO B-groups before the first A-group so the out
                # phase never waits on ACT: B0 B1 A0 B2 A1 ... B7 A6 A7.
                do_b_group(0)
                for g in range(1, NG):
                    do_b_group(g)
                    do_a_group(g - 1)
                    # spread next batch's transposed-layout loads across
                    # this batch's out phase (one quad pair per BA pair)
                    if b + 1 < B_PER_CORE:
                        t_load(b + 1, g - 1)
                do_a_group(NG - 1)
                if b + 1 < B_PER_CORE:
                    t_load(b + 1, NG - 1)

    nc.finalize()
    return nc


def _get_nc():
    if "nc" not in _COMPILED:
        _COMPILED["nc"] = _build()
    return _COMPILED["nc"]


def build_in_maps(x1: np.ndarray, x2: np.ndarray):
    """Host-side shard + layout prep: fp16, tiled transposed + original."""
    Xq = np.asarray(x1, dtype=np.float32).reshape(B, C, D).astype(np.float16)
    Xk = np.asarray(x2, dtype=np.float32).reshape(B, C, D).astype(np.float16)
    ident = np.eye(128, dtype=np.float16)

    def tiled_T(Xb):
        # [bpc, C, D] -> transposed [bpc, D, C] -> quad-tiled [QROWS, 2048]
        # row (b*NQ + j)*128 + p, col s*512 + c  <->  T[b, (j*4+s)*128 + p, c]
        T = Xb.transpose(0, 2, 1).reshape(B_PER_CORE, NQ, 4, 128, C)
        return np.ascontiguousarray(T.transpose(0, 1, 3, 2, 4)).reshape(
            B_PER_CORE * NQ * 128, 4 * C
        )

    in_maps = []
    for i in range(N_CORES):
        sl = slice(i * B_PER_CORE, (i + 1) * B_PER_CORE)
        in_maps.append(
            {
                "qT": tiled_T(Xq[sl]),
                "kT": tiled_T(Xk[sl]),
                "qO": Xq[sl].reshape(B_PER_CORE * C, D),
                "kO": Xk[sl].reshape(B_PER_CORE * C, D),
                "ident": ident,
            }
        )
    return in_maps


def _untile_out(arr):
    # [OROWS, 2048] -> [bpc, C, D]: arr[(b*NG+g)*128+p, cc*512+c] = out[b, cc*128+p, g*512+c]
    t = arr.reshape(B_PER_CORE, NG, 128, CC, 512).transpose(0, 3, 2, 1, 4)
    return t.reshape(B_PER_CORE, C, D)


def kernel(x1: np.ndarray, x2: np.ndarray):
    from concourse.bass_utils import run_bass_kernel_spmd

    nc = _get_nc()
    in_maps = build_in_maps(x1, x2)

    res = None
    for attempt in range(3):
        try:
            res = run_bass_kernel_spmd(nc, in_maps, list(range(N_CORES))).results
            break
        except Exception:
            if attempt == 2:
                raise
    assert res is not None

    outA = np.empty((B, C, 64, 64), dtype=np.float32)
    outB = np.empty((B, C, 64, 64), dtype=np.float32)
    for i in range(N_CORES):
        sl = slice(i * B_PER_CORE, (i + 1) * B_PER_CORE)
        outA[sl] = _untile_out(res[i]["outA"]).astype(np.float32).reshape(
            B_PER_CORE, C, 64, 64
        )
        outB[sl] = _untile_out(res[i]["outB"]).astype(np.float32).reshape(
            B_PER_CORE, C, 64, 64
        )
    return outA, outB
